# revision 15
# baseline (speedup 1.0000x reference)
"""Trainium2 Bass kernel for nn_CustomModel_52484500357175 (GCN message passing).

Reformulated math (biases feeding straight into BatchNorm cancel, since BN
subtracts the per-feature mean):
  s    = segment_sum(x[src], dst)                       # scalar per node
  h1   = relu( s*P + Q  +  aff1b(alter @ C1) )          # C1 = A1a@A1b [6,H]
  agg2 = segment_sum(h1[src], dst)
  h2   = relu( aff2a(agg2 @ W2) + aff2b(alter @ C2) )   # C2 = A2a@A2b
  out  = h2 @ Wl + bl

Key structural points:
  - The alter branches are rank-6 (alter @ C): their BatchNorm statistics
    reduce to the 6x6 gram G = alter^T alter and the column sums of alter.
    mean = (sum_alter @ C)/N; E[z^2]_f = (C^T G C)_ff / N. One tiny AllReduce
    of [7,17] carries G, sum_alter and the s statistics.
  - h1 is produced NODE-major by a single K=8 matmul per 128-node block:
    [alter^T | ones | s] @ [C1*sc ; sh1+Q ; P] -> relu -> bf16 -> DRAM, so
    there is no transpose and no separate alter pass.
  - All matmul operands are bf16 (fp32 matmul is 4x slower on TRN2 PE);
    PSUM accumulation stays fp32. h1 is stored/AllGathered/gathered in bf16,
    halving the dominant HBM/collective traffic.
  - The AllGather of h1 is split along the feature dim (SPLITS pieces) so
    layer-2 gathers+scatter matmuls for piece k overlap the transfer of
    piece k+1.
  - Layer-2 segment_sum: per 128-dst tile an on-chip one-hot
    O[e, d] = (dst_local[e] == iota) feeds PSUM-accumulated matmuls
    agg[fslice] += Gt_fslice.T @ O (feature-major). One-hot matrices for all
    chunks of a tile are built with a single broadcast-AP is_equal op.
"""
import os
import sys

sys.path.insert(0, "/opt/trn_rl_repo")

import numpy as np
import ml_dtypes

import concourse.bass as bass
import concourse.bacc as bacc
import concourse.tile as tile
from concourse import mybir
from concourse import bass_utils

F32 = mybir.dt.float32
BF16 = mybir.dt.bfloat16
I32 = mybir.dt.int32
I16 = mybir.dt.int16
AF = mybir.ActivationFunctionType
OP = mybir.AluOpType
AX = mybir.AxisListType
NPBF = np.dtype(ml_dtypes.bfloat16)

EPS = 1e-5
LO = int(os.environ.get("KERNEL_LO", "32768"))  # int16 range split for gather
SPLITS = int(os.environ.get("KERNEL_SPLITS", "2"))


class Cfg:
    def __init__(self, N=50000, E=500000, H=512, D2=6, OUT=300, NCORES=8):
        self.N, self.E, self.H, self.D2, self.OUT = N, E, H, D2, OUT
        self.NCORES = NCORES
        self.NP = -(-N // (NCORES * 128)) * 128      # per-core nodes
        self.NPAD = self.NP * NCORES
        self.NT = self.NP // 128                     # dst tiles per core
        self.FS = H // 128                           # feature slices
        self.OUTP = -(-OUT // 128) * 128
        self.FO = self.OUTP // 128
        self.chunks = []                             # node chunks <=512 wide
        off = 0
        while off < self.NP:
            w = min(512, self.NP - off)
            self.chunks.append((off, w))
            off += w
        self.NCH = len(self.chunks)


def host_prep(cfg, x, edge_index, alter):
    """Shard edges by destination core/tile; within each 128-dst tile order
    edges [src<LO ..pad.. | src>=LO ..pad..] with per-tile lo/hi chunk counts
    maximized over cores so one SPMD program fits every core. Pad slots carry
    x=0 and dst_local=-1 (their one-hot column is all-zero) and gather row 0.
    """
    c_ = cfg
    src = np.ascontiguousarray(edge_index[0]).astype(np.int64)
    dst = np.ascontiguousarray(edge_index[1]).astype(np.int64)
    x_flat = np.zeros(c_.NPAD, np.float32)
    x_flat[:c_.N] = np.asarray(x, np.float32).ravel()
    owner = dst // c_.NP
    K_lo = np.zeros(c_.NT, np.int64)
    K_hi = np.zeros(c_.NT, np.int64)
    per_core = []
    for c in range(c_.NCORES):
        m = owner == c
        s_c, d_c = src[m], dst[m] - c * c_.NP
        t_c = d_c // 128
        lo_m = s_c < LO
        lists = {}
        for t in range(c_.NT):
            tm = t_c == t
            lists[t] = (s_c[tm & lo_m], d_c[tm & lo_m] - t * 128,
                        s_c[tm & ~lo_m], d_c[tm & ~lo_m] - t * 128)
            K_lo[t] = max(K_lo[t], -(-len(lists[t][0]) // 128))
            K_hi[t] = max(K_hi[t], -(-len(lists[t][2]) // 128))
        per_core.append(lists)
    for t in range(c_.NT):
        if K_lo[t] == 0 and K_hi[t] == 0:
            K_lo[t] = 1
    K = K_lo + K_hi
    TOTK = int(K.sum())
    col0 = np.concatenate([[0], np.cumsum(K)])[:-1].astype(np.int64)
    SIDX = TOTK * 8

    xe = np.zeros((c_.NCORES, 128, TOTK), NPBF)
    dl = np.full((c_.NCORES, 128, TOTK), -1.0, NPBF)
    idx16 = np.zeros((c_.NCORES, 128, SIDX), np.int16)
    for c in range(c_.NCORES):
        lists = per_core[c]
        for t in range(c_.NT):
            s_lo, d_lo, s_hi, d_hi = lists[t]
            for (s_l, d_l, Kh, coff) in (
                    (s_lo, d_lo, int(K_lo[t]), int(col0[t])),
                    (s_hi - LO, d_hi, int(K_hi[t]), int(col0[t] + K_lo[t]))):
                if Kh == 0:
                    continue
                n = len(s_l)
                nidx = Kh * 128
                a16 = np.zeros(nidx, np.int16)
                a16[:n] = s_l.astype(np.int16)
                idx16[c, :, coff * 8:(coff + Kh) * 8] = np.tile(
                    a16.reshape(nidx // 16, 16).T, (8, 1))
                dlv = np.full(nidx, -1.0, np.float32)
                dlv[:n] = d_l.astype(np.float32)
                dl[c, :, coff:coff + Kh] = dlv.reshape(Kh, 128).T.astype(NPBF)
            # x values (absolute src ids; pad slots carry 0.0)
            for (s_a, Kh, coff) in (
                    (s_lo, int(K_lo[t]), int(col0[t])),
                    (s_hi, int(K_hi[t]), int(col0[t] + K_lo[t]))):
                if Kh == 0:
                    continue
                n = len(s_a)
                nidx = Kh * 128
                xv = np.zeros(nidx, np.float32)
                xv[:n] = x_flat[s_a.astype(np.int64)]
                xe[c, :, coff:coff + Kh] = xv.reshape(Kh, 128).T.astype(NPBF)

    alt8 = np.zeros((c_.NCORES, 8, c_.NP), NPBF)
    altnm = np.zeros((c_.NCORES, 128, c_.NT * 7), NPBF)
    for c in range(c_.NCORES):
        rows = np.asarray(alter[c * c_.NP:min((c + 1) * c_.NP, c_.N)], np.float32)
        alt8[c, 0:c_.D2, :rows.shape[0]] = rows.T.astype(NPBF)
        alt8[c, 6, :] = 1.0
        for t in range(c_.NT):
            r0 = c * c_.NP + t * 128
            r1 = min(r0 + 128, c_.N)
            nn_ = max(0, r1 - r0)
            if nn_ > 0:
                altnm[c, :nn_, t * 7:t * 7 + 6] = np.asarray(
                    alter[r0:r1], np.float32).astype(NPBF)
            altnm[c, :, t * 7 + 6] = 1.0
    return dict(TOTK=TOTK, SIDX=SIDX, K_lo=K_lo, K_hi=K_hi, col0=col0,
                xe=xe, dl=dl, idx16=idx16, alt8=alt8, altnm=altnm)


def build_program(cfg, prep):
    _ph = os.environ.get("KERNEL_PHASE", "4")
    DO_AG = _ph != "1"
    DO_GATHER = _ph in ("22", "3", "4")
    DO_SCATTER = _ph in ("3", "4")
    DO_FIN = _ph == "4"
    REPEAT = int(os.environ.get("KERNEL_REPEAT", "1"))
    c_ = cfg
    TOTK, SIDX = prep["TOTK"], prep["SIDX"]
    K_lo, K_hi, col0 = prep["K_lo"], prep["K_hi"], prep["col0"]
    FS, NT, NP, OUTP, FO, NCH = c_.FS, c_.NT, c_.NP, c_.OUTP, c_.FO, c_.NCH
    H, D2 = c_.H, c_.D2
    HS = H // SPLITS                 # features per AllGather split
    FSS = FS // SPLITS               # feature slices per split
    invN = 1.0 / c_.N
    rg = [list(range(c_.NCORES))]
    KMAXT = int((K_lo + K_hi).max())

    nc = bacc.Bacc("TRN2", target_bir_lowering=False, debug=False,
                   enable_asserts=False, num_devices=c_.NCORES)

    d_xe = nc.dram_tensor("xe", [128, TOTK], BF16, kind="ExternalInput")
    d_dl = nc.dram_tensor("dl", [128, TOTK], BF16, kind="ExternalInput")
    d_idx = nc.dram_tensor("idx16", [128, SIDX], I16, kind="ExternalInput")
    d_alt8 = nc.dram_tensor("alt8", [8, NP], BF16, kind="ExternalInput")
    d_altnm = nc.dram_tensor("altnm", [128, NT * 7], BF16, kind="ExternalInput")
    d_a1at = nc.dram_tensor("a1at", [128, FS * D2], BF16, kind="ExternalInput")
    d_a1b = nc.dram_tensor("a1b", [128, FS * H], BF16, kind="ExternalInput")
    d_a2at = nc.dram_tensor("a2at", [128, FS * D2], BF16, kind="ExternalInput")
    d_a2b = nc.dram_tensor("a2b", [128, FS * H], BF16, kind="ExternalInput")
    d_w2t = nc.dram_tensor("w2t", [128, FS * H], BF16, kind="ExternalInput")
    d_wlt = nc.dram_tensor("wlt", [128, FS * OUTP], BF16, kind="ExternalInput")
    d_bnr = nc.dram_tensor("bnr", [1, 7 * H], F32, kind="ExternalInput")
    d_bnc = nc.dram_tensor("bnc", [128, 2 * FS + FO], F32, kind="ExternalInput")
    d_out = nc.dram_tensor("outT", [OUTP, NP], BF16, kind="ExternalOutput")

    shared = "Shared" if c_.NCORES > 4 else "Local"

    import contextlib
    with tile.TileContext(nc) as tc, contextlib.ExitStack() as ctx:
        dpool = ctx.enter_context(tc.tile_pool(name="dram", bufs=1, space="DRAM"))
        d_h1nm = [dpool.tile([NP, HS], BF16, name=f"h1nm{sp}")
                  for sp in range(SPLITS)]
        d_h1full = [dpool.tile([c_.NPAD, HS], BF16, name=f"h1full{sp}",
                               addr_space=shared)
                    for sp in range(SPLITS)]
        d_ar1i = dpool.tile([7, 18], F32, name="ar1i")
        d_ar1o = dpool.tile([7, 18], F32, name="ar1o", addr_space=shared)
        d_ar2i = dpool.tile([128, 2 * FS], F32, name="ar2i")
        d_ar2o = dpool.tile([128, 2 * FS], F32, name="ar2o", addr_space=shared)

        cst = ctx.enter_context(tc.tile_pool(name="cst", bufs=1))
        # ---------------- constants / weights ----------------
        iota_i = cst.tile([128, 128], I32)
        nc.gpsimd.iota(iota_i[:], pattern=[[1, 128]], base=0, channel_multiplier=0)
        iota_bf = cst.tile([128, 128], BF16)
        nc.vector.tensor_copy(iota_bf[:], iota_i[:])
        ones6 = cst.tile([6, 1], F32)
        nc.vector.memset(ones6[:], 1.0)

        sb_xe = cst.tile([128, TOTK], BF16)
        nc.sync.dma_start(sb_xe[:], d_xe[:])
        sb_dl = cst.tile([128, TOTK], BF16)
        nc.sync.dma_start(sb_dl[:], d_dl[:])
        sb_idx = cst.tile([128, SIDX], I16)
        nc.sync.dma_start(sb_idx[:], d_idx[:])
        sb_alt8 = cst.tile([8, NP], BF16)
        nc.sync.dma_start(sb_alt8[:], d_alt8[:])
        sb_altnm = cst.tile([128, NT * 7], BF16)
        nc.sync.dma_start(sb_altnm[:], d_altnm[:])
        sb_a1at = cst.tile([128, FS * D2], BF16)
        nc.sync.dma_start(sb_a1at[:], d_a1at[:])
        sb_a1b = cst.tile([128, FS * H], BF16)
        nc.sync.dma_start(sb_a1b[:], d_a1b[:])
        sb_a2at = cst.tile([128, FS * D2], BF16)
        nc.sync.dma_start(sb_a2at[:], d_a2at[:])
        sb_a2b = cst.tile([128, FS * H], BF16)
        nc.sync.dma_start(sb_a2b[:], d_a2b[:])
        sb_w2t = cst.tile([128, FS * H], BF16)
        nc.sync.dma_start(sb_w2t[:], d_w2t[:])
        sb_wlt = cst.tile([128, FS * OUTP], BF16)
        nc.sync.dma_start(sb_wlt[:], d_wlt[:])
        sb_bnr = cst.tile([1, 7 * H], F32)
        nc.sync.dma_start(sb_bnr[:], d_bnr[:])
        sb_bnc = cst.tile([128, 2 * FS + FO], F32)
        nc.sync.dma_start(sb_bnc[:], d_bnc[:])

        def bnr_row(i):
            return sb_bnr[:, i * H:(i + 1) * H]
        g1a_r, be1a_r = bnr_row(0), bnr_row(1)
        g1b_r, be1b_r = bnr_row(2), bnr_row(3)
        g2b_r, be2b_r = bnr_row(4), bnr_row(5)
        w1_r = bnr_row(6)

        # persistent row/param storage
        s_bfrow = cst.tile([1, NP], BF16)
        ssum_row = cst.tile([1, NT + 2], F32)
        ssq_row = cst.tile([1, NT + 2], F32)
        sqscr = cst.tile([1, 128], F32)
        arG = cst.tile([7, 18], F32)
        nc.vector.memset(arG[:], 0.0)
        sbC1 = cst.tile([6, H], F32)
        sbC2 = cst.tile([6, H], F32)
        rhs8_1 = cst.tile([8, H], BF16)      # C1*sc1 ; sh1+Q ; P
        lhs7_2 = cst.tile([7, H], BF16)      # C2*sc2b ; sh2b
        rowp = cst.tile([1, 6 * H], F32)     # row scratch
        rowbf = cst.tile([1, 2 * H], BF16)
        scbc = cst.tile([7, H], F32)
        st_z = cst.tile([128, 2 * FS * NCH], F32)
        ar2 = cst.tile([128, 2 * FS], F32)
        prm2 = cst.tile([128, 2 * FS], F32)  # sc2a | sh2a columns
        tmp8 = cst.tile([128, 8], F32)
        rrow = cst.tile([1, H], F32)         # 1/sc2a row (from column transpose)
        trs = cst.tile([FS, 128], F32)       # transpose staging
        aggs = [cst.tile([128, NP], BF16, name=f"aggs_{fs}") for fs in range(FS)]

        def R(i):
            return rowp[:, i * H:(i + 1) * H]

        def onehots(op_pool, t):
            """one [128, kt*128] bf16 one-hot stack for all chunks of tile t"""
            kt = int(K_lo[t] + K_hi[t])
            c0 = int(col0[t])
            O = op_pool.tile([128, KMAXT * 128], BF16, tag="O", name="O_big")
            nc.vector.tensor_tensor(
                out=O[:, :kt * 128].rearrange("p (c e) -> p c e", e=128),
                in0=iota_bf[:].unsqueeze(1).broadcast_to([128, kt, 128]),
                in1=sb_dl[:, c0:c0 + kt].unsqueeze(2).broadcast_to([128, kt, 128]),
                op=OP.is_equal)
            return O, kt, c0

        for _rep in range(REPEAT):
            # ============ layer 1: s = segment_sum(x[src]) ============
            with tc.tile_pool(name="psS", bufs=2, space="PSUM") as psS, \
                 tc.tile_pool(name="psG", bufs=1, space="PSUM") as psG, \
                 tc.tile_pool(name="op1", bufs=3) as op1:
                psg = psG.tile([7, 16], F32, name="ps_G")
                for t in range(NT):
                    O, kt, c0 = onehots(op1, t)
                    ps = psS.tile([1, 512], F32, tag="s", name="ps_s")
                    for j in range(kt):
                        nc.tensor.matmul(out=ps[:, :128],
                                         lhsT=sb_xe[:, c0 + j:c0 + j + 1],
                                         rhs=O[:, j * 128:(j + 1) * 128],
                                         start=(j == 0), stop=(j == kt - 1))
                    nc.scalar.activation(s_bfrow[:, t * 128:(t + 1) * 128],
                                         ps[:, :128], AF.Copy,
                                         accum_out=ssum_row[:, t:t + 1])
                    nc.scalar.activation(sqscr[:], ps[:, :128], AF.Square,
                                         accum_out=ssq_row[:, t:t + 1])
                    # gram of [alter | 1] (accumulated over node blocks)
                    nc.tensor.matmul(out=psg[:, :7],
                                     lhsT=sb_altnm[:, t * 7:t * 7 + 7],
                                     rhs=sb_altnm[:, t * 7:t * 7 + 7],
                                     start=(t == 0), stop=(t == NT - 1))
                # s row -> partition 7 of alt8 (for the fused h1 matmul)
                nc.sync.dma_start(sb_alt8[7:8, :], s_bfrow[:])
                # pack AR1 payload [7,17]: G7 | col16 row0-1 = sum_s, sumsq_s
                nc.scalar.activation(arG[:, 0:7], psg[:, 0:7], AF.Copy)
                nc.vector.tensor_reduce(arG[0:1, 16:17], ssum_row[:, :NT],
                                        axis=AX.X, op=OP.add)
                nc.vector.tensor_reduce(arG[0:1, 17:18], ssq_row[:, :NT],
                                        axis=AX.X, op=OP.add)

            # C1 = A1a @ A1b, C2 = A2a @ A2b  (tiny, overlaps scatter above)
            with tc.tile_pool(name="psC", bufs=2, space="PSUM") as psC:
                for (at, ab, dst_c) in ((sb_a1at, sb_a1b, sbC1),
                                        (sb_a2at, sb_a2b, sbC2)):
                    pc = psC.tile([6, H], F32, tag="C", name="ps_C")
                    for s4 in range(FS):
                        nc.tensor.matmul(out=pc[:],
                                         lhsT=at[:, s4 * D2:(s4 + 1) * D2],
                                         rhs=ab[:, s4 * H:(s4 + 1) * H],
                                         start=(s4 == 0), stop=(s4 == FS - 1))
                    nc.scalar.activation(dst_c[:], pc[:], AF.Copy)

            # ============ AllReduce 1 ============
            nc.sync.dma_start(d_ar1i[:], arG[:])
            nc.gpsimd.collective_compute("AllReduce", OP.add, replica_groups=rg,
                                         ins=[d_ar1i[:]], outs=[d_ar1o[:]])
            nc.sync.dma_start(arG[:], d_ar1o[:])

            # ============ BN params layer 1 (rows) ============
            def branch_rows(sbC, g_row, be_row, sc_out_f32):
                """sc/sh rows for a rank-6 branch: stats from gram G."""
                with tc.tile_pool(name="psR", bufs=1, space="PSUM") as psR:
                    pt = psR.tile([6, H], F32, tag="T", name="ps_T")
                    nc.tensor.matmul(out=pt[:], lhsT=arG[0:6, 0:6], rhs=sbC[:],
                                     start=True, stop=True)
                    ct = cst.tile([6, H], F32, tag="ct", name="ct6") \
                        if False else None
                    nc.vector.tensor_tensor(out=scbc[:], in0=sbC[:], in1=pt[:],
                                            op=OP.mult)
                    pm = psR.tile([1, H], F32, tag="M2", name="ps_M2")
                    nc.tensor.matmul(out=pm[:], lhsT=ones6[:], rhs=scbc[:],
                                     start=True, stop=True)
                    pm1 = psR.tile([1, H], F32, tag="m1", name="ps_m1")
                    nc.tensor.matmul(out=pm1[:], lhsT=arG[0:6, 6:7], rhs=sbC[:],
                                     start=True, stop=True)
                    m, M2, v, t = R(0), R(1), R(2), R(3)
                    nc.vector.tensor_scalar(out=m, in0=pm1[:], scalar1=invN,
                                            scalar2=None, op0=OP.mult)
                    nc.vector.tensor_scalar(out=M2, in0=pm[:], scalar1=invN,
                                            scalar2=None, op0=OP.mult)
                nc.vector.tensor_tensor(out=t, in0=m, in1=m, op=OP.mult)
                nc.vector.tensor_tensor(out=v, in0=M2, in1=t, op=OP.subtract)
                nc.vector.tensor_scalar(out=v, in0=v, scalar1=EPS, scalar2=None,
                                        op0=OP.add)
                nc.scalar.activation(v, v, AF.Sqrt)
                nc.vector.reciprocal(t, v)
                sc = sc_out_f32
                nc.vector.tensor_tensor(out=sc, in0=t, in1=g_row, op=OP.mult)
                sh = R(4)
                nc.vector.tensor_tensor(out=t, in0=m, in1=sc, op=OP.mult)
                nc.vector.tensor_tensor(out=sh, in0=be_row, in1=t, op=OP.subtract)
                return sc, sh

            sc1, sh1 = branch_rows(sbC1, g1b_r, be1b_r, R(5))
            # BN1a (GCN branch): P = W1*g1a/sqrt(W1^2*var_s+eps), Q = be1a-m_s*P
            ms, vs = tmp8[0:1, 0:1], tmp8[0:1, 1:2]
            nc.vector.tensor_scalar(out=ms, in0=arG[0:1, 16:17], scalar1=invN,
                                    scalar2=None, op0=OP.mult)
            nc.vector.tensor_scalar(out=vs, in0=arG[0:1, 17:18], scalar1=invN,
                                    scalar2=None, op0=OP.mult)
            t2 = tmp8[0:1, 2:3]
            nc.vector.tensor_tensor(out=t2, in0=ms, in1=ms, op=OP.mult)
            nc.vector.tensor_tensor(out=vs, in0=vs, in1=t2, op=OP.subtract)
            tP, tQ, tw = R(0), R(1), R(2)
            nc.vector.tensor_tensor(out=tw, in0=w1_r, in1=w1_r, op=OP.mult)
            nc.vector.tensor_scalar(out=tw, in0=tw, scalar1=vs, scalar2=EPS,
                                    op0=OP.mult, op1=OP.add)
            nc.scalar.activation(tw, tw, AF.Sqrt)
            nc.vector.reciprocal(tw, tw)
            nc.vector.tensor_tensor(out=tP, in0=tw, in1=w1_r, op=OP.mult)
            nc.vector.tensor_tensor(out=tP, in0=tP, in1=g1a_r, op=OP.mult)
            nc.vector.tensor_scalar(out=tQ, in0=tP, scalar1=ms, scalar2=None,
                                    op0=OP.mult)
            nc.vector.tensor_tensor(out=tQ, in0=be1a_r, in1=tQ, op=OP.subtract)
            # SH1 = sh1 + Q ; to bf16 rows, then into rhs8_1 partitions 6,7
            nc.vector.tensor_tensor(out=tQ, in0=tQ, in1=sh1, op=OP.add)
            nc.vector.tensor_copy(rowbf[:, 0:H], tQ)
            nc.vector.tensor_copy(rowbf[:, H:2 * H], tP)
            nc.gpsimd.partition_broadcast(scbc[:], sc1, channels=6)
            nc.vector.tensor_tensor(out=rhs8_1[0:6, :], in0=sbC1[:], in1=scbc[:],
                                    op=OP.mult)
            nc.sync.dma_start(rhs8_1[6:7, :], rowbf[:, 0:H])
            nc.sync.dma_start(rhs8_1[7:8, :], rowbf[:, H:2 * H])

            # ============ h1 finalize (node-major, one matmul per block) ======
            with tc.tile_pool(name="psH", bufs=3, space="PSUM") as psH, \
                 tc.tile_pool(name="hbf", bufs=3) as hbf:
                for t in range(NT):
                    ph = psH.tile([128, H], F32, tag="h", name="ps_h1")
                    nc.tensor.matmul(out=ph[:],
                                     lhsT=sb_alt8[0:8, t * 128:(t + 1) * 128],
                                     rhs=rhs8_1[:], start=True, stop=True)
                    hb = hbf.tile([128, H], BF16, tag="h", name="h1_bf")
                    nc.scalar.activation(hb[:], ph[:], AF.Relu)
                    for sp in range(SPLITS):
                        nc.sync.dma_start(
                            d_h1nm[sp][t * 128:(t + 1) * 128, :],
                            hb[:, sp * HS:(sp + 1) * HS])

            # ============ AllGather h1 (split along features) ============
            if DO_AG:
                for sp in range(SPLITS):
                    nc.gpsimd.collective_compute(
                        "AllGather", OP.bypass, replica_groups=rg,
                        ins=[d_h1nm[sp][:]], outs=[d_h1full[sp][:]])

            # BN2b rows + C2aff during the AllGather window
            sc2, sh2 = branch_rows(sbC2, g2b_r, be2b_r, R(5))
            nc.vector.tensor_copy(rowbf[:, 0:H], sh2)
            nc.gpsimd.partition_broadcast(scbc[:], sc2, channels=6)
            nc.vector.tensor_tensor(out=lhs7_2[0:6, :], in0=sbC2[:], in1=scbc[:],
                                    op=OP.mult)
            nc.sync.dma_start(lhs7_2[6:7, :], rowbf[:, 0:H])

            # ============ layer 2: gather + scatter (per AllGather split) =====
            # split sp's gathers/scatter run while split sp+1 is still in
            # flight on the collective; agg slices persist in SBUF (bf16).
            if DO_GATHER:
                n_hi = c_.NPAD - LO
                with tc.tile_pool(name="gp", bufs=3) as gp, \
                     tc.tile_pool(name="op2", bufs=3) as op2, \
                     tc.tile_pool(name="psA", bufs=3, space="PSUM") as psA, \
                     tc.tile_pool(name="psB", bufs=2, space="PSUM") as psB, \
                     tc.tile_pool(name="wk2", bufs=2) as wk2:
                    for sp in range(SPLITS):
                        for ncid, (off, w) in enumerate(c_.chunks):
                            for t in range(off // 128, (off + w) // 128):
                                kt = int(K_lo[t] + K_hi[t])
                                c0 = int(col0[t])
                                Gt = gp.tile([128, KMAXT * HS], BF16,
                                             tag="G", name="Gt")
                                for (koff, Kh, base, nrows) in (
                                        (0, int(K_lo[t]), 0, min(LO, c_.NPAD)),
                                        (int(K_lo[t]), int(K_hi[t]), LO, n_hi)):
                                    if Kh == 0:
                                        continue
                                    nidx = Kh * 128
                                    ic0 = (c0 + koff) * 8
                                    nc.gpsimd.dma_gather(
                                        out_ap=Gt[:, koff * HS:(koff + Kh) * HS]
                                        .rearrange("p (c e) -> p c e", e=HS),
                                        in_ap=d_h1full[sp][base:base + nrows, :],
                                        idxs_ap=sb_idx[:, ic0:ic0 + nidx // 16],
                                        num_idxs=nidx, num_idxs_reg=nidx,
                                        elem_size=HS)
                                if not DO_SCATTER:
                                    nc.vector.tensor_reduce(
                                        st_z[:, t:t + 1], Gt[:, :kt * HS],
                                        axis=AX.X, op=OP.add)
                                    continue
                                O, _, _ = onehots(op2, t)
                                for f2 in range(FSS):
                                    fs = sp * FSS + f2
                                    ps_sc = psA.tile([128, 512], F32, tag="acc",
                                                     name="ps_sc")
                                    for j in range(kt):
                                        nc.tensor.matmul(
                                            out=ps_sc[:, :128],
                                            lhsT=Gt[:, j * HS + f2 * 128:
                                                    j * HS + f2 * 128 + 128],
                                            rhs=O[:, j * 128:(j + 1) * 128],
                                            start=(j == 0), stop=(j == kt - 1))
                                    nc.scalar.activation(
                                        aggs[fs][:, t * 128:(t + 1) * 128],
                                        ps_sc[:, :128], AF.Copy)
                            if not DO_SCATTER or sp != SPLITS - 1:
                                continue
                            # all agg slices ready -> z2 statistics (streaming)
                            for fo in range(FS):
                                pd = psB.tile([128, 512], F32, tag="z",
                                              name="ps_z2")
                                for fi in range(FS):
                                    nc.tensor.matmul(
                                        out=pd[:, :w],
                                        lhsT=sb_w2t[:, fi * H + fo * 128:
                                                    fi * H + (fo + 1) * 128],
                                        rhs=aggs[fi][:, off:off + w],
                                        start=(fi == 0), stop=(fi == FS - 1))
                                nc.vector.tensor_reduce(
                                    st_z[:, fo * NCH + ncid:fo * NCH + ncid + 1],
                                    pd[:, :w], axis=AX.X, op=OP.add)
                                sq = wk2.tile([128, 512], F32, tag="sq",
                                              name="sq_z")
                                nc.scalar.activation(
                                    sq[:, :w], pd[:, :w], AF.Square,
                                    accum_out=st_z[:, (FS + fo) * NCH + ncid:
                                                   (FS + fo) * NCH + ncid + 1])

            if DO_FIN:
                # ============ AllReduce 2 + BN2a params (columns) ============
                for q in range(2 * FS):
                    nc.vector.tensor_reduce(ar2[:, q:q + 1],
                                            st_z[:, q * NCH:(q + 1) * NCH],
                                            axis=AX.X, op=OP.add)
                nc.sync.dma_start(d_ar2i[:], ar2[:])
                nc.gpsimd.collective_compute("AllReduce", OP.add,
                                             replica_groups=rg,
                                             ins=[d_ar2i[:]], outs=[d_ar2o[:]])
                nc.sync.dma_start(ar2[:], d_ar2o[:])
                for fo in range(FS):
                    m, v, r = tmp8[:, 0:1], tmp8[:, 1:2], tmp8[:, 2:3]
                    nc.vector.tensor_scalar(out=m, in0=ar2[:, fo:fo + 1],
                                            scalar1=invN, scalar2=None,
                                            op0=OP.mult)
                    nc.vector.tensor_scalar(out=v, in0=ar2[:, FS + fo:FS + fo + 1],
                                            scalar1=invN, scalar2=None,
                                            op0=OP.mult)
                    nc.vector.tensor_tensor(out=r, in0=m, in1=m, op=OP.mult)
                    nc.vector.tensor_tensor(out=v, in0=v, in1=r, op=OP.subtract)
                    nc.vector.tensor_scalar(out=v, in0=v, scalar1=EPS,
                                            scalar2=None, op0=OP.add)
                    nc.scalar.activation(v, v, AF.Sqrt)
                    nc.vector.reciprocal(r, v)
                    nc.vector.tensor_tensor(out=prm2[:, fo:fo + 1], in0=r,
                                            in1=sb_bnc[:, fo:fo + 1], op=OP.mult)
                    nc.vector.tensor_tensor(out=r, in0=m, in1=prm2[:, fo:fo + 1],
                                            op=OP.mult)
                    nc.vector.tensor_tensor(out=prm2[:, FS + fo:FS + fo + 1],
                                            in0=sb_bnc[:, FS + fo:FS + fo + 1],
                                            in1=r, op=OP.subtract)
                # fold 1/sc2a into lhs7_2 so h2 = relu(psum*sc2a + sh2a)
                # with a single fused PSUM group + activation per (chunk, fo)
                with tc.tile_pool(name="psR2", bufs=1, space="PSUM") as psR2:
                    nc.vector.reciprocal(tmp8[:, 3:3 + FS], prm2[:, 0:FS])
                    ptr = psR2.tile([FS, 128], F32, name="ps_tr")
                    nc.tensor.transpose(out=ptr[:], in_=tmp8[:, 3:3 + FS],
                                        identity=ident[:])
                    nc.scalar.activation(trs[:], ptr[:], AF.Copy)
                    for i in range(FS):
                        nc.sync.dma_start(rrow[:, i * 128:(i + 1) * 128],
                                          trs[i:i + 1, :])
                nc.gpsimd.partition_broadcast(scbc[:], rrow[:], channels=7)
                nc.vector.tensor_tensor(out=lhs7_2[:], in0=lhs7_2[:],
                                        in1=scbc[0:7, :], op=OP.mult)

                # ============ h2 + head ============
                with tc.tile_pool(name="psF", bufs=3, space="PSUM") as psF, \
                     tc.tile_pool(name="h2p", bufs=2) as h2p, \
                     tc.tile_pool(name="wkf", bufs=3) as wkf:
                    for ncid, (off, w) in enumerate(c_.chunks):
                        hs2 = []
                        for fo in range(FS):
                            pv = psF.tile([128, 512], F32, tag="pv", name="ps_v2")
                            nc.tensor.matmul(
                                out=pv[:, :w],
                                lhsT=lhs7_2[:, fo * 128:(fo + 1) * 128],
                                rhs=sb_alt8[0:7, off:off + w],
                                start=True, stop=False)
                            for fi in range(FS):
                                nc.tensor.matmul(
                                    out=pv[:, :w],
                                    lhsT=sb_w2t[:, fi * H + fo * 128:
                                                fi * H + (fo + 1) * 128],
                                    rhs=aggs[fi][:, off:off + w],
                                    start=False, stop=(fi == FS - 1))
                            hb = h2p.tile([128, 512], BF16, tag=f"h{fo}",
                                          name=f"h2_{fo}")
                            nc.scalar.activation(hb[:, :w], pv[:, :w], AF.Relu,
                                                 scale=prm2[:, fo:fo + 1],
                                                 bias=prm2[:, FS + fo:FS + fo + 1])
                            hs2.append(hb)
                        for fo in range(FO):
                            po = psF.tile([128, 512], F32, tag="pv", name="ps_o")
                            for fi in range(FS):
                                nc.tensor.matmul(
                                    out=po[:, :w],
                                    lhsT=sb_wlt[:, fi * OUTP + fo * 128:
                                                fi * OUTP + (fo + 1) * 128],
                                    rhs=hs2[fi][:, :w],
                                    start=(fi == 0), stop=(fi == FS - 1))
                            ot = wkf.tile([128, 512], BF16, tag="ot", name="ot")
                            nc.vector.tensor_scalar(
                                out=ot[:, :w], in0=po[:, :w],
                                scalar1=sb_bnc[:, 2 * FS + fo:2 * FS + fo + 1],
                                scalar2=None, op0=OP.add)
                            nc.sync.dma_start(
                                d_out[fo * 128:(fo + 1) * 128, off:off + w],
                                ot[:, :w])

    nc.compile()
    return nc


def make_inputs(cfg, prep, params, core):
    c_ = cfg
    FS, H, D2, OUTP, FO = c_.FS, c_.H, c_.D2, c_.OUTP, c_.FO

    def blocks_T(M, cols):
        """[H, cols] -> [128, FS*cols] with block s = M[s*128:(s+1)*128, :]"""
        out = np.zeros((128, FS * cols), NPBF)
        for s in range(FS):
            out[:, s * cols:(s + 1) * cols] = M[s * 128:(s + 1) * 128, :].astype(NPBF)
        return out

    A1aT = np.asarray(params["A1a"], np.float32).T          # [H, 6]
    A2aT = np.asarray(params["A2a"], np.float32).T
    Wl_pad = np.zeros((H, OUTP), np.float32)
    Wl_pad[:, :c_.OUT] = params["Wl"]
    bl_pad = np.zeros(OUTP, np.float32)
    bl_pad[:c_.OUT] = params["bl"]
    bnr = np.concatenate([params[k].astype(np.float32).ravel() for k in
                          ("g1a", "be1a", "g1b", "be1b", "g2b", "be2b")] +
                         [params["W1"].astype(np.float32).ravel()])[None, :]
    bnc = np.zeros((128, 2 * FS + FO), np.float32)
    bnc[:, 0:FS] = params["g2a"].reshape(FS, 128).T
    bnc[:, FS:2 * FS] = params["be2a"].reshape(FS, 128).T
    bnc[:, 2 * FS:] = bl_pad.reshape(FO, 128).T
    return {
        "xe": np.ascontiguousarray(prep["xe"][core]),
        "dl": np.ascontiguousarray(prep["dl"][core]),
        "idx16": np.ascontiguousarray(prep["idx16"][core]),
        "alt8": np.ascontiguousarray(prep["alt8"][core]),
        "altnm": np.ascontiguousarray(prep["altnm"][core]),
        "a1at": blocks_T(A1aT, D2), "a1b": blocks_T(params["A1b"], H),
        "a2at": blocks_T(A2aT, D2), "a2b": blocks_T(params["A2b"], H),
        "w2t": blocks_T(params["W2"], H), "wlt": blocks_T(Wl_pad, OUTP),
        "bnr": bnr, "bnc": bnc,
    }


_CACHE = {}


def kernel(**inputs):
    cfg = Cfg()
    x = np.asarray(inputs["x"], np.float32)
    ei = np.asarray(inputs["edge_index"])
    alter = np.asarray(inputs["alter_edge_attr"], np.float32)
    params = {k: np.asarray(v, np.float32) for k, v in inputs.items()
              if k not in ("x", "edge_index", "alter_edge_attr")}
    prep = host_prep(cfg, x, ei, alter)

    key = (prep["TOTK"], prep["K_lo"].tobytes(), prep["K_hi"].tobytes())
    if key not in _CACHE:
        _CACHE[key] = build_program(cfg, prep)
    nc = _CACHE[key]

    in_maps = [make_inputs(cfg, prep, params, c) for c in range(cfg.NCORES)]
    res = bass_utils.run_bass_kernel_spmd(nc, in_maps, core_ids=list(range(cfg.NCORES)))
    chunks = [np.asarray(res.results[c]["outT"]).astype(np.float32).T
              for c in range(cfg.NCORES)]
    full = np.concatenate(chunks, axis=0)
    return np.ascontiguousarray(full[:cfg.N, :cfg.OUT]).astype(np.float32)


# revision 20
# speedup vs baseline: 1.6796x; 1.6796x over previous
"""Trainium2 Bass kernel for nn_CustomModel_52484500357175 (GCN message passing).

Reformulated math (biases feeding straight into BatchNorm cancel, since BN
subtracts the per-feature mean):
  s    = segment_sum(x[src], dst)                       # scalar per node
  h1   = relu( s*P + Q  +  aff1b(alter @ C1) )          # C1 = A1a@A1b [6,H]
  agg2 = segment_sum(h1[src], dst)
  h2   = relu( aff2a(agg2 @ W2) + aff2b(alter @ C2) )   # C2 = A2a@A2b
  out  = h2 @ Wl + bl

Key structural points:
  - The alter branches are rank-6 (alter @ C): their BatchNorm statistics
    reduce to the 6x6 gram G = alter^T alter and the column sums of alter.
    mean = (sum_alter @ C)/N; E[z^2]_f = (C^T G C)_ff / N. One tiny AllReduce
    of [7,17] carries G, sum_alter and the s statistics.
  - h1 is produced NODE-major by a single K=8 matmul per 128-node block:
    [alter^T | ones | s] @ [C1*sc ; sh1+Q ; P] -> relu -> bf16 -> DRAM, so
    there is no transpose and no separate alter pass.
  - All matmul operands are bf16 (fp32 matmul is 4x slower on TRN2 PE);
    PSUM accumulation stays fp32. h1 is stored/AllGathered/gathered in bf16,
    halving the dominant HBM/collective traffic.
  - The AllGather of h1 is split along the feature dim (SPLITS pieces) so
    layer-2 gathers+scatter matmuls for piece k overlap the transfer of
    piece k+1.
  - Layer-2 segment_sum: per 128-dst tile an on-chip one-hot
    O[e, d] = (dst_local[e] == iota) feeds PSUM-accumulated matmuls
    agg[fslice] += Gt_fslice.T @ O (feature-major). One-hot matrices for all
    chunks of a tile are built with a single broadcast-AP is_equal op.
"""
import os
import sys

sys.path.insert(0, "/opt/trn_rl_repo")

import numpy as np
import ml_dtypes

import concourse.bass as bass
import concourse.bacc as bacc
import concourse.tile as tile
from concourse import mybir
from concourse import bass_utils
from concourse.masks import make_identity

F32 = mybir.dt.float32
BF16 = mybir.dt.float16   # 16-bit compute dtype (fp16: more mantissa than bf16)
I32 = mybir.dt.int32
I16 = mybir.dt.int16
AF = mybir.ActivationFunctionType
OP = mybir.AluOpType
AX = mybir.AxisListType
NPBF = np.dtype(np.float16)

EPS = 1e-5
LO = int(os.environ.get("KERNEL_LO", "32768"))  # int16 range split for gather
SPLITS = int(os.environ.get("KERNEL_SPLITS", "2"))
H1DT = mybir.dt.float8e4 if os.environ.get("KERNEL_H1DT", "fp16") == "fp8" \
    else mybir.dt.float16  # dtype of h1 store / AllGather / gather path


class Cfg:
    def __init__(self, N=50000, E=500000, H=512, D2=6, OUT=300, NCORES=8):
        self.N, self.E, self.H, self.D2, self.OUT = N, E, H, D2, OUT
        self.NCORES = NCORES
        self.NP = -(-N // (NCORES * 128)) * 128      # per-core nodes
        self.NPAD = self.NP * NCORES
        self.NT = self.NP // 128                     # dst tiles per core
        self.FS = H // 128                           # feature slices
        self.OUTP = -(-OUT // 128) * 128
        self.FO = self.OUTP // 128
        self.chunks = []                             # node chunks <=512 wide
        off = 0
        while off < self.NP:
            w = min(512, self.NP - off)
            self.chunks.append((off, w))
            off += w
        self.NCH = len(self.chunks)


def host_prep(cfg, x, edge_index, alter):
    """Shard edges by destination core/tile; within each 128-dst tile order
    edges [src<LO ..pad.. | src>=LO ..pad..] with per-tile lo/hi chunk counts
    maximized over cores so one SPMD program fits every core. Pad slots carry
    x=0 and dst_local=-1 (their one-hot column is all-zero) and gather row 0.
    """
    c_ = cfg
    src = np.ascontiguousarray(edge_index[0]).astype(np.int64)
    dst = np.ascontiguousarray(edge_index[1]).astype(np.int64)
    x_flat = np.zeros(c_.NPAD, np.float32)
    x_flat[:c_.N] = np.asarray(x, np.float32).ravel()
    owner = dst // c_.NP
    K_lo = np.zeros(c_.NT, np.int64)
    K_hi = np.zeros(c_.NT, np.int64)
    per_core = []
    for c in range(c_.NCORES):
        m = owner == c
        s_c, d_c = src[m], dst[m] - c * c_.NP
        t_c = d_c // 128
        lo_m = s_c < LO
        lists = {}
        for t in range(c_.NT):
            tm = t_c == t
            lists[t] = (s_c[tm & lo_m], d_c[tm & lo_m] - t * 128,
                        s_c[tm & ~lo_m], d_c[tm & ~lo_m] - t * 128)
            K_lo[t] = max(K_lo[t], -(-len(lists[t][0]) // 128))
            K_hi[t] = max(K_hi[t], -(-len(lists[t][2]) // 128))
        per_core.append(lists)
    for t in range(c_.NT):
        if K_lo[t] == 0 and K_hi[t] == 0:
            K_lo[t] = 1
    K = K_lo + K_hi
    TOTK = int(K.sum())
    col0 = np.concatenate([[0], np.cumsum(K)])[:-1].astype(np.int64)
    SIDX = TOTK * 8

    xe = np.zeros((c_.NCORES, 128, TOTK), NPBF)
    dl = np.full((c_.NCORES, 128, TOTK), -1.0, NPBF)
    idx16 = np.zeros((c_.NCORES, 128, SIDX), np.int16)
    for c in range(c_.NCORES):
        lists = per_core[c]
        for t in range(c_.NT):
            s_lo, d_lo, s_hi, d_hi = lists[t]
            for (s_l, d_l, Kh, coff) in (
                    (s_lo, d_lo, int(K_lo[t]), int(col0[t])),
                    (s_hi - LO, d_hi, int(K_hi[t]), int(col0[t] + K_lo[t]))):
                if Kh == 0:
                    continue
                n = len(s_l)
                nidx = Kh * 128
                a16 = np.zeros(nidx, np.int16)
                a16[:n] = s_l.astype(np.int16)
                idx16[c, :, coff * 8:(coff + Kh) * 8] = np.tile(
                    a16.reshape(nidx // 16, 16).T, (8, 1))
                dlv = np.full(nidx, -1.0, np.float32)
                dlv[:n] = d_l.astype(np.float32)
                dl[c, :, coff:coff + Kh] = dlv.reshape(Kh, 128).T.astype(NPBF)
            # x values (absolute src ids; pad slots carry 0.0)
            for (s_a, Kh, coff) in (
                    (s_lo, int(K_lo[t]), int(col0[t])),
                    (s_hi, int(K_hi[t]), int(col0[t] + K_lo[t]))):
                if Kh == 0:
                    continue
                n = len(s_a)
                nidx = Kh * 128
                xv = np.zeros(nidx, np.float32)
                xv[:n] = x_flat[s_a.astype(np.int64)]
                xe[c, :, coff:coff + Kh] = xv.reshape(Kh, 128).T.astype(NPBF)

    alt8 = np.zeros((c_.NCORES, 8, c_.NP), NPBF)
    altnm = np.zeros((c_.NCORES, 128, c_.NT * 7), NPBF)
    for c in range(c_.NCORES):
        rows = np.asarray(alter[c * c_.NP:min((c + 1) * c_.NP, c_.N)], np.float32)
        alt8[c, 0:c_.D2, :rows.shape[0]] = rows.T.astype(NPBF)
        alt8[c, 6, :] = 1.0
        for t in range(c_.NT):
            r0 = c * c_.NP + t * 128
            r1 = min(r0 + 128, c_.N)
            nn_ = max(0, r1 - r0)
            if nn_ > 0:
                altnm[c, :nn_, t * 7:t * 7 + 6] = np.asarray(
                    alter[r0:r1], np.float32).astype(NPBF)
            altnm[c, :, t * 7 + 6] = 1.0
    return dict(TOTK=TOTK, SIDX=SIDX, K_lo=K_lo, K_hi=K_hi, col0=col0,
                xe=xe, dl=dl, idx16=idx16, alt8=alt8, altnm=altnm)


def build_program(cfg, prep):
    _ph = os.environ.get("KERNEL_PHASE", "4")
    DO_AG = _ph != "1"
    DO_GATHER = _ph in ("22", "3", "4")
    DO_SCATTER = _ph in ("3", "4")
    DO_FIN = _ph == "4"
    REPEAT = int(os.environ.get("KERNEL_REPEAT", "1"))
    c_ = cfg
    TOTK, SIDX = prep["TOTK"], prep["SIDX"]
    K_lo, K_hi, col0 = prep["K_lo"], prep["K_hi"], prep["col0"]
    FS, NT, NP, OUTP, FO, NCH = c_.FS, c_.NT, c_.NP, c_.OUTP, c_.FO, c_.NCH
    H, D2 = c_.H, c_.D2
    HS = H // SPLITS                 # features per AllGather split
    FSS = FS // SPLITS               # feature slices per split
    invN = 1.0 / c_.N
    rg = [list(range(c_.NCORES))]
    KMAXT = int((K_lo + K_hi).max())

    nc = bacc.Bacc("TRN2", target_bir_lowering=False, debug=False,
                   enable_asserts=False, num_devices=c_.NCORES)

    d_xe = nc.dram_tensor("xe", [128, TOTK], BF16, kind="ExternalInput")
    d_dl = nc.dram_tensor("dl", [128, TOTK], BF16, kind="ExternalInput")
    d_idx = nc.dram_tensor("idx16", [128, SIDX], I16, kind="ExternalInput")
    d_alt8 = nc.dram_tensor("alt8", [8, NP], BF16, kind="ExternalInput")
    d_altnm = nc.dram_tensor("altnm", [128, NT * 7], BF16, kind="ExternalInput")
    d_a1at = nc.dram_tensor("a1at", [128, FS * D2], BF16, kind="ExternalInput")
    d_a1b = nc.dram_tensor("a1b", [128, FS * H], BF16, kind="ExternalInput")
    d_a2at = nc.dram_tensor("a2at", [128, FS * D2], BF16, kind="ExternalInput")
    d_a2b = nc.dram_tensor("a2b", [128, FS * H], BF16, kind="ExternalInput")
    d_w2t = nc.dram_tensor("w2t", [128, FS * H], BF16, kind="ExternalInput")
    d_wlt = nc.dram_tensor("wlt", [128, FS * OUTP], BF16, kind="ExternalInput")
    d_bnr = nc.dram_tensor("bnr", [1, 7 * H], F32, kind="ExternalInput")
    d_bnc = nc.dram_tensor("bnc", [128, 2 * FS + FO], F32, kind="ExternalInput")
    d_out = nc.dram_tensor("outT", [OUTP, NP], BF16, kind="ExternalOutput")

    shared = "Shared" if c_.NCORES > 4 else "Local"

    import contextlib
    with tile.TileContext(nc) as tc, contextlib.ExitStack() as ctx:
        dpool = ctx.enter_context(tc.tile_pool(name="dram", bufs=1, space="DRAM"))
        d_h1nm = [dpool.tile([NP, HS], H1DT, name=f"h1nm{sp}")
                  for sp in range(SPLITS)]
        d_h1full = [dpool.tile([c_.NPAD, HS], H1DT, name=f"h1full{sp}",
                               addr_space=shared)
                    for sp in range(SPLITS)]
        d_ar1i = dpool.tile([7, 18], F32, name="ar1i")
        d_ar1o = dpool.tile([7, 18], F32, name="ar1o", addr_space=shared)
        d_ar2i = dpool.tile([128, 2 * FS], F32, name="ar2i")
        d_ar2o = dpool.tile([128, 2 * FS], F32, name="ar2o", addr_space=shared)

        cst = ctx.enter_context(tc.tile_pool(name="cst", bufs=1))
        # ---------------- constants / weights ----------------
        iota_i = cst.tile([128, 128], I32)
        nc.gpsimd.iota(iota_i[:], pattern=[[1, 128]], base=0, channel_multiplier=0)
        iota_bf = cst.tile([128, 128], BF16)
        nc.vector.tensor_copy(iota_bf[:], iota_i[:])
        ident = cst.tile([128, 128], F32)
        make_identity(nc, ident[:])
        ones6 = cst.tile([6, 1], F32)
        nc.vector.memset(ones6[:], 1.0)

        sb_xe = cst.tile([128, TOTK], BF16)
        nc.sync.dma_start(sb_xe[:], d_xe[:])
        sb_dl = cst.tile([128, TOTK], BF16)
        nc.sync.dma_start(sb_dl[:], d_dl[:])
        sb_idx = cst.tile([128, SIDX], I16)
        nc.sync.dma_start(sb_idx[:], d_idx[:])
        sb_alt8 = cst.tile([8, NP], BF16)
        nc.sync.dma_start(sb_alt8[:], d_alt8[:])
        sb_altnm = cst.tile([128, NT * 7], BF16)
        nc.sync.dma_start(sb_altnm[:], d_altnm[:])
        sb_a1at = cst.tile([128, FS * D2], BF16)
        nc.sync.dma_start(sb_a1at[:], d_a1at[:])
        sb_a1b = cst.tile([128, FS * H], BF16)
        nc.sync.dma_start(sb_a1b[:], d_a1b[:])
        sb_a2at = cst.tile([128, FS * D2], BF16)
        nc.sync.dma_start(sb_a2at[:], d_a2at[:])
        sb_a2b = cst.tile([128, FS * H], BF16)
        nc.sync.dma_start(sb_a2b[:], d_a2b[:])
        sb_w2t = cst.tile([128, FS * H], BF16)
        nc.sync.dma_start(sb_w2t[:], d_w2t[:])
        sb_wlt = cst.tile([128, FS * OUTP], BF16)
        nc.sync.dma_start(sb_wlt[:], d_wlt[:])
        sb_bnr = cst.tile([1, 7 * H], F32)
        nc.sync.dma_start(sb_bnr[:], d_bnr[:])
        sb_bnc = cst.tile([128, 2 * FS + FO], F32)
        nc.sync.dma_start(sb_bnc[:], d_bnc[:])

        def bnr_row(i):
            return sb_bnr[:, i * H:(i + 1) * H]
        g1a_r, be1a_r = bnr_row(0), bnr_row(1)
        g1b_r, be1b_r = bnr_row(2), bnr_row(3)
        g2b_r, be2b_r = bnr_row(4), bnr_row(5)
        w1_r = bnr_row(6)

        # persistent row/param storage
        s_bfrow = cst.tile([1, NP], BF16)
        ssum_row = cst.tile([1, NT + 2], F32)
        ssq_row = cst.tile([1, NT + 2], F32)
        sqscr = cst.tile([1, 128], F32)
        arG = cst.tile([7, 18], F32)
        nc.vector.memset(arG[:], 0.0)
        sbC1 = cst.tile([6, H], F32)
        sbC2 = cst.tile([6, H], F32)
        rhs8_1 = cst.tile([8, H], BF16)      # C1*sc1 ; sh1+Q ; P
        lhs7_2 = cst.tile([7, H], BF16)      # C2*sc2b ; sh2b
        rowp = cst.tile([1, 6 * H], F32)     # row scratch
        rowbf = cst.tile([1, 2 * H], BF16)
        scbc = cst.tile([7, H], F32)
        st_z = cst.tile([128, 2 * FS * NCH], F32)
        ar2 = cst.tile([128, 2 * FS], F32)
        prm2 = cst.tile([128, 2 * FS], F32)  # sc2a | sh2a columns
        tmp8 = cst.tile([128, 8], F32)
        rrow = cst.tile([1, H], F32)         # 1/sc2a row (from column transpose)
        trs = cst.tile([FS, 128], F32)       # transpose staging
        aggs = [cst.tile([128, NP], BF16, name=f"aggs_{fs}") for fs in range(FS)]

        def R(i):
            return rowp[:, i * H:(i + 1) * H]

        def onehots(op_pool, t, dt=BF16):
            """one [128, kt*128] one-hot stack for all chunks of tile t"""
            kt = int(K_lo[t] + K_hi[t])
            c0 = int(col0[t])
            O = op_pool.tile([128, KMAXT * 128], dt, tag="O", name="O_big")
            nc.vector.tensor_tensor(
                out=O[:, :kt * 128].rearrange("p (c e) -> p c e", e=128),
                in0=iota_bf[:].unsqueeze(1).broadcast_to([128, kt, 128]),
                in1=sb_dl[:, c0:c0 + kt].unsqueeze(2).broadcast_to([128, kt, 128]),
                op=OP.is_equal)
            return O, kt, c0

        for _rep in range(REPEAT):
            # ============ layer 1: s = segment_sum(x[src]) ============
            with tc.tile_pool(name="psS", bufs=2, space="PSUM") as psS, \
                 tc.tile_pool(name="psG", bufs=1, space="PSUM") as psG, \
                 tc.tile_pool(name="op1", bufs=3) as op1:
                psg = psG.tile([7, 16], F32, name="ps_G")
                for t in range(NT):
                    O, kt, c0 = onehots(op1, t)
                    ps = psS.tile([1, 512], F32, tag="s", name="ps_s")
                    for j in range(kt):
                        nc.tensor.matmul(out=ps[:, :128],
                                         lhsT=sb_xe[:, c0 + j:c0 + j + 1],
                                         rhs=O[:, j * 128:(j + 1) * 128],
                                         start=(j == 0), stop=(j == kt - 1))
                    nc.scalar.activation(s_bfrow[:, t * 128:(t + 1) * 128],
                                         ps[:, :128], AF.Copy,
                                         accum_out=ssum_row[:, t:t + 1])
                    nc.scalar.activation(sqscr[:], ps[:, :128], AF.Square,
                                         accum_out=ssq_row[:, t:t + 1])
                    # gram of [alter | 1] (accumulated over node blocks)
                    nc.tensor.matmul(out=psg[:, :7],
                                     lhsT=sb_altnm[:, t * 7:t * 7 + 7],
                                     rhs=sb_altnm[:, t * 7:t * 7 + 7],
                                     start=(t == 0), stop=(t == NT - 1))
                # s row -> partition 7 of alt8 (for the fused h1 matmul)
                nc.sync.dma_start(sb_alt8[7:8, :], s_bfrow[:])
                # pack AR1 payload [7,17]: G7 | col16 row0-1 = sum_s, sumsq_s
                nc.scalar.activation(arG[:, 0:7], psg[:, 0:7], AF.Copy)
                nc.vector.tensor_reduce(arG[0:1, 16:17], ssum_row[:, :NT],
                                        axis=AX.X, op=OP.add)
                nc.vector.tensor_reduce(arG[0:1, 17:18], ssq_row[:, :NT],
                                        axis=AX.X, op=OP.add)

            # C1 = A1a @ A1b, C2 = A2a @ A2b  (tiny, overlaps scatter above)
            with tc.tile_pool(name="psC", bufs=2, space="PSUM") as psC:
                for (at, ab, dst_c) in ((sb_a1at, sb_a1b, sbC1),
                                        (sb_a2at, sb_a2b, sbC2)):
                    pc = psC.tile([6, H], F32, tag="C", name="ps_C")
                    for s4 in range(FS):
                        nc.tensor.matmul(out=pc[:],
                                         lhsT=at[:, s4 * D2:(s4 + 1) * D2],
                                         rhs=ab[:, s4 * H:(s4 + 1) * H],
                                         start=(s4 == 0), stop=(s4 == FS - 1))
                    nc.scalar.activation(dst_c[:], pc[:], AF.Copy)

            # ============ AllReduce 1 ============
            nc.sync.dma_start(d_ar1i[:], arG[:])
            nc.gpsimd.collective_compute("AllReduce", OP.add, replica_groups=rg,
                                         ins=[d_ar1i[:]], outs=[d_ar1o[:]])
            nc.sync.dma_start(arG[:], d_ar1o[:])

            # ============ BN params layer 1 (rows) ============
            def branch_rows(sbC, g_row, be_row, sc_out_f32):
                """sc/sh rows for a rank-6 branch: stats from gram G."""
                with tc.tile_pool(name="psR", bufs=1, space="PSUM") as psR:
                    pt = psR.tile([6, H], F32, tag="T", name="ps_T")
                    nc.tensor.matmul(out=pt[:], lhsT=arG[0:6, 0:6], rhs=sbC[:],
                                     start=True, stop=True)
                    ct = cst.tile([6, H], F32, tag="ct", name="ct6") \
                        if False else None
                    nc.vector.tensor_tensor(out=scbc[0:6, :], in0=sbC[:], in1=pt[:],
                                            op=OP.mult)
                    pm = psR.tile([1, H], F32, tag="M2", name="ps_M2")
                    nc.tensor.matmul(out=pm[:], lhsT=ones6[:], rhs=scbc[0:6, :],
                                     start=True, stop=True)
                    pm1 = psR.tile([1, H], F32, tag="m1", name="ps_m1")
                    nc.tensor.matmul(out=pm1[:], lhsT=arG[0:6, 6:7], rhs=sbC[:],
                                     start=True, stop=True)
                    m, M2, v, t = R(0), R(1), R(2), R(3)
                    nc.vector.tensor_scalar(out=m, in0=pm1[:], scalar1=invN,
                                            scalar2=None, op0=OP.mult)
                    nc.vector.tensor_scalar(out=M2, in0=pm[:], scalar1=invN,
                                            scalar2=None, op0=OP.mult)
                nc.vector.tensor_tensor(out=t, in0=m, in1=m, op=OP.mult)
                nc.vector.tensor_tensor(out=v, in0=M2, in1=t, op=OP.subtract)
                nc.vector.tensor_scalar(out=v, in0=v, scalar1=EPS, scalar2=None,
                                        op0=OP.add)
                nc.scalar.activation(v, v, AF.Sqrt)
                nc.vector.reciprocal(t, v)
                sc = sc_out_f32
                nc.vector.tensor_tensor(out=sc, in0=t, in1=g_row, op=OP.mult)
                sh = R(4)
                nc.vector.tensor_tensor(out=t, in0=m, in1=sc, op=OP.mult)
                nc.vector.tensor_tensor(out=sh, in0=be_row, in1=t, op=OP.subtract)
                return sc, sh

            sc1, sh1 = branch_rows(sbC1, g1b_r, be1b_r, R(5))
            # BN1a (GCN branch): P = W1*g1a/sqrt(W1^2*var_s+eps), Q = be1a-m_s*P
            ms, vs = tmp8[0:1, 0:1], tmp8[0:1, 1:2]
            nc.vector.tensor_scalar(out=ms, in0=arG[0:1, 16:17], scalar1=invN,
                                    scalar2=None, op0=OP.mult)
            nc.vector.tensor_scalar(out=vs, in0=arG[0:1, 17:18], scalar1=invN,
                                    scalar2=None, op0=OP.mult)
            t2 = tmp8[0:1, 2:3]
            nc.vector.tensor_tensor(out=t2, in0=ms, in1=ms, op=OP.mult)
            nc.vector.tensor_tensor(out=vs, in0=vs, in1=t2, op=OP.subtract)
            tP, tQ, tw = R(0), R(1), R(2)
            nc.vector.tensor_tensor(out=tw, in0=w1_r, in1=w1_r, op=OP.mult)
            nc.vector.tensor_scalar(out=tw, in0=tw, scalar1=vs, scalar2=EPS,
                                    op0=OP.mult, op1=OP.add)
            nc.scalar.activation(tw, tw, AF.Sqrt)
            nc.vector.reciprocal(tw, tw)
            nc.vector.tensor_tensor(out=tP, in0=tw, in1=w1_r, op=OP.mult)
            nc.vector.tensor_tensor(out=tP, in0=tP, in1=g1a_r, op=OP.mult)
            nc.vector.tensor_scalar(out=tQ, in0=tP, scalar1=ms, scalar2=None,
                                    op0=OP.mult)
            nc.vector.tensor_tensor(out=tQ, in0=be1a_r, in1=tQ, op=OP.subtract)
            # SH1 = sh1 + Q ; to bf16 rows, then into rhs8_1 partitions 6,7
            nc.vector.tensor_tensor(out=tQ, in0=tQ, in1=sh1, op=OP.add)
            nc.vector.tensor_copy(rowbf[:, 0:H], tQ)
            nc.vector.tensor_copy(rowbf[:, H:2 * H], tP)
            nc.gpsimd.partition_broadcast(scbc[0:6, :], sc1, channels=6)
            nc.vector.tensor_tensor(out=rhs8_1[0:6, :], in0=sbC1[:],
                                    in1=scbc[0:6, :], op=OP.mult)
            nc.sync.dma_start(rhs8_1[6:7, :], rowbf[:, 0:H])
            nc.sync.dma_start(rhs8_1[7:8, :], rowbf[:, H:2 * H])

            # ============ h1 finalize (node-major, one matmul per block) ======
            with tc.tile_pool(name="psH", bufs=3, space="PSUM") as psH, \
                 tc.tile_pool(name="hbf", bufs=3) as hbf:
                for t in range(NT):
                    ph = psH.tile([128, H], F32, tag="h", name="ps_h1")
                    nc.tensor.matmul(out=ph[:],
                                     lhsT=sb_alt8[0:8, t * 128:(t + 1) * 128],
                                     rhs=rhs8_1[:], start=True, stop=True)
                    hb = hbf.tile([128, H], H1DT, tag="h", name="h1_bf")
                    nc.scalar.activation(hb[:], ph[:], AF.Relu)
                    for sp in range(SPLITS):
                        nc.sync.dma_start(
                            d_h1nm[sp][t * 128:(t + 1) * 128, :],
                            hb[:, sp * HS:(sp + 1) * HS])

            # ============ AllGather h1 (split along features) ============
            if DO_AG:
                for sp in range(SPLITS):
                    nc.gpsimd.collective_compute(
                        "AllGather", OP.bypass, replica_groups=rg,
                        ins=[d_h1nm[sp][:]], outs=[d_h1full[sp][:]])

            # BN2b rows + C2aff during the AllGather window
            sc2, sh2 = branch_rows(sbC2, g2b_r, be2b_r, R(5))
            nc.vector.tensor_copy(rowbf[:, 0:H], sh2)
            nc.gpsimd.partition_broadcast(scbc[0:6, :], sc2, channels=6)
            nc.vector.tensor_tensor(out=lhs7_2[0:6, :], in0=sbC2[:],
                                    in1=scbc[0:6, :], op=OP.mult)
            nc.sync.dma_start(lhs7_2[6:7, :], rowbf[:, 0:H])

            # ============ layer 2: gather + scatter (per AllGather split) =====
            # split sp's gathers/scatter run while split sp+1 is still in
            # flight on the collective; agg slices persist in SBUF (bf16).
            if DO_GATHER:
                n_hi = c_.NPAD - LO
                with tc.tile_pool(name="gp", bufs=3) as gp, \
                     tc.tile_pool(name="op2", bufs=3) as op2, \
                     tc.tile_pool(name="psA", bufs=3, space="PSUM") as psA, \
                     tc.tile_pool(name="psB", bufs=2, space="PSUM") as psB, \
                     tc.tile_pool(name="wk2", bufs=2) as wk2:
                    for sp in range(SPLITS):
                        for ncid, (off, w) in enumerate(c_.chunks):
                            for t in range(off // 128, (off + w) // 128):
                                kt = int(K_lo[t] + K_hi[t])
                                c0 = int(col0[t])
                                Gt = gp.tile([128, KMAXT * HS], H1DT,
                                             tag="G", name="Gt")
                                for (koff, Kh, base, nrows) in (
                                        (0, int(K_lo[t]), 0, min(LO, c_.NPAD)),
                                        (int(K_lo[t]), int(K_hi[t]), LO, n_hi)):
                                    if Kh == 0:
                                        continue
                                    nidx = Kh * 128
                                    ic0 = (c0 + koff) * 8
                                    nc.gpsimd.dma_gather(
                                        out_ap=Gt[:, koff * HS:(koff + Kh) * HS]
                                        .rearrange("p (c e) -> p c e", e=HS),
                                        in_ap=d_h1full[sp][base:base + nrows, :],
                                        idxs_ap=sb_idx[:, ic0:ic0 + nidx // 16],
                                        num_idxs=nidx, num_idxs_reg=nidx,
                                        elem_size=HS)
                                if not DO_SCATTER:
                                    nc.vector.tensor_reduce(
                                        st_z[:, t:t + 1], Gt[:, :kt * HS],
                                        axis=AX.X, op=OP.add)
                                    continue
                                O, _, _ = onehots(op2, t, dt=H1DT)
                                for f2 in range(FSS):
                                    fs = sp * FSS + f2
                                    ps_sc = psA.tile([128, 512], F32, tag="acc",
                                                     name="ps_sc")
                                    for j in range(kt):
                                        nc.tensor.matmul(
                                            out=ps_sc[:, :128],
                                            lhsT=Gt[:, j * HS + f2 * 128:
                                                    j * HS + f2 * 128 + 128],
                                            rhs=O[:, j * 128:(j + 1) * 128],
                                            start=(j == 0), stop=(j == kt - 1))
                                    nc.scalar.activation(
                                        aggs[fs][:, t * 128:(t + 1) * 128],
                                        ps_sc[:, :128], AF.Copy)
                            if not DO_SCATTER or sp != SPLITS - 1:
                                continue
                            # all agg slices ready -> z2 statistics (streaming)
                            for fo in range(FS):
                                pd = psB.tile([128, 512], F32, tag="z",
                                              name="ps_z2")
                                for fi in range(FS):
                                    nc.tensor.matmul(
                                        out=pd[:, :w],
                                        lhsT=sb_w2t[:, fi * H + fo * 128:
                                                    fi * H + (fo + 1) * 128],
                                        rhs=aggs[fi][:, off:off + w],
                                        start=(fi == 0), stop=(fi == FS - 1))
                                nc.vector.tensor_reduce(
                                    st_z[:, fo * NCH + ncid:fo * NCH + ncid + 1],
                                    pd[:, :w], axis=AX.X, op=OP.add)
                                sq = wk2.tile([128, 512], F32, tag="sq",
                                              name="sq_z")
                                nc.scalar.activation(
                                    sq[:, :w], pd[:, :w], AF.Square,
                                    accum_out=st_z[:, (FS + fo) * NCH + ncid:
                                                   (FS + fo) * NCH + ncid + 1])

            if DO_FIN:
                # ============ AllReduce 2 + BN2a params (columns) ============
                for q in range(2 * FS):
                    nc.vector.tensor_reduce(ar2[:, q:q + 1],
                                            st_z[:, q * NCH:(q + 1) * NCH],
                                            axis=AX.X, op=OP.add)
                nc.sync.dma_start(d_ar2i[:], ar2[:])
                nc.gpsimd.collective_compute("AllReduce", OP.add,
                                             replica_groups=rg,
                                             ins=[d_ar2i[:]], outs=[d_ar2o[:]])
                nc.sync.dma_start(ar2[:], d_ar2o[:])
                for fo in range(FS):
                    m, v, r = tmp8[:, 0:1], tmp8[:, 1:2], tmp8[:, 2:3]
                    nc.vector.tensor_scalar(out=m, in0=ar2[:, fo:fo + 1],
                                            scalar1=invN, scalar2=None,
                                            op0=OP.mult)
                    nc.vector.tensor_scalar(out=v, in0=ar2[:, FS + fo:FS + fo + 1],
                                            scalar1=invN, scalar2=None,
                                            op0=OP.mult)
                    nc.vector.tensor_tensor(out=r, in0=m, in1=m, op=OP.mult)
                    nc.vector.tensor_tensor(out=v, in0=v, in1=r, op=OP.subtract)
                    nc.vector.tensor_scalar(out=v, in0=v, scalar1=EPS,
                                            scalar2=None, op0=OP.add)
                    nc.scalar.activation(v, v, AF.Sqrt)
                    nc.vector.reciprocal(r, v)
                    nc.vector.tensor_tensor(out=prm2[:, fo:fo + 1], in0=r,
                                            in1=sb_bnc[:, fo:fo + 1], op=OP.mult)
                    nc.vector.tensor_tensor(out=r, in0=m, in1=prm2[:, fo:fo + 1],
                                            op=OP.mult)
                    nc.vector.tensor_tensor(out=prm2[:, FS + fo:FS + fo + 1],
                                            in0=sb_bnc[:, FS + fo:FS + fo + 1],
                                            in1=r, op=OP.subtract)
                # fold 1/sc2a into lhs7_2 so h2 = relu(psum*sc2a + sh2a)
                # with a single fused PSUM group + activation per (chunk, fo)
                with tc.tile_pool(name="psR2", bufs=1, space="PSUM") as psR2:
                    nc.vector.reciprocal(tmp8[:, 3:3 + FS], prm2[:, 0:FS])
                    ptr = psR2.tile([FS, 128], F32, name="ps_tr")
                    nc.tensor.transpose(out=ptr[:], in_=tmp8[:, 3:3 + FS],
                                        identity=ident[:])
                    nc.scalar.activation(trs[:], ptr[:], AF.Copy)
                    for i in range(FS):
                        nc.sync.dma_start(rrow[:, i * 128:(i + 1) * 128],
                                          trs[i:i + 1, :])
                nc.gpsimd.partition_broadcast(scbc[:], rrow[:], channels=7)
                nc.vector.tensor_tensor(out=lhs7_2[:], in0=lhs7_2[:],
                                        in1=scbc[0:7, :], op=OP.mult)

                # ============ h2 + head ============
                with tc.tile_pool(name="psF", bufs=3, space="PSUM") as psF, \
                     tc.tile_pool(name="h2p", bufs=2) as h2p, \
                     tc.tile_pool(name="wkf", bufs=3) as wkf:
                    for ncid, (off, w) in enumerate(c_.chunks):
                        hs2 = []
                        for fo in range(FS):
                            pv = psF.tile([128, 512], F32, tag="pv", name="ps_v2")
                            nc.tensor.matmul(
                                out=pv[:, :w],
                                lhsT=lhs7_2[:, fo * 128:(fo + 1) * 128],
                                rhs=sb_alt8[0:7, off:off + w],
                                start=True, stop=False)
                            for fi in range(FS):
                                nc.tensor.matmul(
                                    out=pv[:, :w],
                                    lhsT=sb_w2t[:, fi * H + fo * 128:
                                                fi * H + (fo + 1) * 128],
                                    rhs=aggs[fi][:, off:off + w],
                                    start=False, stop=(fi == FS - 1))
                            hb = h2p.tile([128, 512], BF16, tag=f"h{fo}",
                                          name=f"h2_{fo}")
                            nc.scalar.activation(hb[:, :w], pv[:, :w], AF.Relu,
                                                 scale=prm2[:, fo:fo + 1],
                                                 bias=prm2[:, FS + fo:FS + fo + 1])
                            hs2.append(hb)
                        for fo in range(FO):
                            po = psF.tile([128, 512], F32, tag="pv", name="ps_o")
                            for fi in range(FS):
                                nc.tensor.matmul(
                                    out=po[:, :w],
                                    lhsT=sb_wlt[:, fi * OUTP + fo * 128:
                                                fi * OUTP + (fo + 1) * 128],
                                    rhs=hs2[fi][:, :w],
                                    start=(fi == 0), stop=(fi == FS - 1))
                            ot = wkf.tile([128, 512], BF16, tag="ot", name="ot")
                            nc.vector.tensor_scalar(
                                out=ot[:, :w], in0=po[:, :w],
                                scalar1=sb_bnc[:, 2 * FS + fo:2 * FS + fo + 1],
                                scalar2=None, op0=OP.add)
                            nc.sync.dma_start(
                                d_out[fo * 128:(fo + 1) * 128, off:off + w],
                                ot[:, :w])

    nc.compile()
    return nc


def make_inputs(cfg, prep, params, core):
    c_ = cfg
    FS, H, D2, OUTP, FO = c_.FS, c_.H, c_.D2, c_.OUTP, c_.FO

    def blocks_T(M, cols):
        """[H, cols] -> [128, FS*cols] with block s = M[s*128:(s+1)*128, :]"""
        out = np.zeros((128, FS * cols), NPBF)
        for s in range(FS):
            out[:, s * cols:(s + 1) * cols] = M[s * 128:(s + 1) * 128, :].astype(NPBF)
        return out

    A1aT = np.asarray(params["A1a"], np.float32).T          # [H, 6]
    A2aT = np.asarray(params["A2a"], np.float32).T
    Wl_pad = np.zeros((H, OUTP), np.float32)
    Wl_pad[:, :c_.OUT] = params["Wl"]
    bl_pad = np.zeros(OUTP, np.float32)
    bl_pad[:c_.OUT] = params["bl"]
    bnr = np.concatenate([params[k].astype(np.float32).ravel() for k in
                          ("g1a", "be1a", "g1b", "be1b", "g2b", "be2b")] +
                         [params["W1"].astype(np.float32).ravel()])[None, :]
    bnc = np.zeros((128, 2 * FS + FO), np.float32)
    bnc[:, 0:FS] = params["g2a"].reshape(FS, 128).T
    bnc[:, FS:2 * FS] = params["be2a"].reshape(FS, 128).T
    bnc[:, 2 * FS:] = bl_pad.reshape(FO, 128).T
    return {
        "xe": np.ascontiguousarray(prep["xe"][core]),
        "dl": np.ascontiguousarray(prep["dl"][core]),
        "idx16": np.ascontiguousarray(prep["idx16"][core]),
        "alt8": np.ascontiguousarray(prep["alt8"][core]),
        "altnm": np.ascontiguousarray(prep["altnm"][core]),
        "a1at": blocks_T(A1aT, D2), "a1b": blocks_T(params["A1b"], H),
        "a2at": blocks_T(A2aT, D2), "a2b": blocks_T(params["A2b"], H),
        "w2t": blocks_T(params["W2"], H), "wlt": blocks_T(Wl_pad, OUTP),
        "bnr": bnr, "bnc": bnc,
    }


_CACHE = {}


def kernel(**inputs):
    cfg = Cfg()
    x = np.asarray(inputs["x"], np.float32)
    ei = np.asarray(inputs["edge_index"])
    alter = np.asarray(inputs["alter_edge_attr"], np.float32)
    params = {k: np.asarray(v, np.float32) for k, v in inputs.items()
              if k not in ("x", "edge_index", "alter_edge_attr")}
    prep = host_prep(cfg, x, ei, alter)

    key = (prep["TOTK"], prep["K_lo"].tobytes(), prep["K_hi"].tobytes())
    if key not in _CACHE:
        _CACHE[key] = build_program(cfg, prep)
    nc = _CACHE[key]

    in_maps = [make_inputs(cfg, prep, params, c) for c in range(cfg.NCORES)]
    res = bass_utils.run_bass_kernel_spmd(nc, in_maps, core_ids=list(range(cfg.NCORES)))
    chunks = [np.asarray(res.results[c]["outT"]).astype(np.float32).T
              for c in range(cfg.NCORES)]
    full = np.concatenate(chunks, axis=0)
    return np.ascontiguousarray(full[:cfg.N, :cfg.OUT]).astype(np.float32)
